# revision 1
# baseline (speedup 1.0000x reference)
"""Trainium2 Bass kernel for multi-head causal attention with RoPE.

Problem: x[4,2048,1024] -> MHA(16 heads, head_dim 64, RoPE, causal) -> [4,2048,1024]

Sharding: 8 cores = 4 batches x 2 head-groups (8 heads each, Megatron-style).
Each core computes a partial [T, C] projection output for its batch; the host
sums the two head-group partials per batch and adds b_proj.

Per-core dataflow (all on-device):
  - x^T via PE transposes
  - Q^T/K^T computed in [c', t] layout (head-pair tiles of 128 partitions),
    RoPE fused on the PSUM->SBUF path using host-precomputed cos/sin tables
    (1/sqrt(64) folded into W_q on host)
  - V in natural [t, c'] layout (bf16, head-major with a spare column)
  - scores S^T = K Q^T per (head pair, 512-wide q chunk, 128-wide k chunk),
    both heads row-tiled into one [128,1024] PSUM tile; causal block skipping;
    one batched exp on ACT per tile (max-subtraction-free softmax - scores are
    O(+-6) so exp is safe); diagonal blocks masked after exp (bf16 P)
  - P@V in O^T form: V as stationary, col-tiled per head into [128(2h x 64d),
    512q] PSUM; denominators via parallel ones-matmuls into a row-aligned
    [128, 512] PSUM tile; normalize = one reciprocal + one multiply, writing
    y^T directly (no transposes back)
  - output projection y^T @ W_proj accumulated over head pairs (fp32r)
"""

import math
import sys

import numpy as np

if "/opt/trn_rl_repo" not in sys.path:
    sys.path.insert(0, "/opt/trn_rl_repo")

import concourse.bass as bass
import concourse.tile as tile
from concourse import bacc
from concourse import mybir
from concourse.bass_utils import run_bass_kernel_spmd
from concourse.masks import make_identity

B, T, C = 4, 2048, 1024
NH, D = 16, 64
HL = 8              # local heads per core
DL = HL * D         # 512
NCORES = 8
P = 128
TCH = 512           # t-chunk width in phase A
NTC = T // TCH
ROPE_BASE = 10000.0

F32 = mybir.dt.float32
F32R = mybir.dt.float32r
BF16 = mybir.dt.bfloat16
Exp = mybir.ActivationFunctionType.Exp


def _emit(tc, xb, wqk, wv, wp, cos2, sin2, bias, mask, perm, out):
    import os
    phases = os.environ.get("K_PHASES", "abc")
    nc = tc.nc
    with tc.tile_pool(name="pers", bufs=1) as pers:
        qkT = pers.tile([P, 8, T], F32R)          # j 0-3: Q pairs, 4-7: K pairs
        vsb = pers.tile([P, 16, HL * 65], BF16)   # [t mod 128, t tile, h*65 + e]
        ident = pers.tile([P, P], F32)
        make_identity(nc, ident)

        # ---------------- Phase A: x^T, Q^T/K^T (+RoPE), V ----------------
        with tc.tile_pool(name="pha", bufs=1) as pa, \
             tc.tile_pool(name="stage", bufs=4) as pstg, \
             tc.tile_pool(name="tmp", bufs=2) as ptmp, \
             tc.tile_pool(name="psA", bufs=3, space="PSUM") as psA, \
             tc.tile_pool(name="psq", bufs=2, space="PSUM") as psQ, \
             tc.tile_pool(name="psw", bufs=1, space="PSUM") as psW, \
             tc.tile_pool(name="psv", bufs=2, space="PSUM") as psV:
            wqk_sb = pa.tile([P, 8, 2 * DL], F32R)
            nc.gpsimd.dma_start(wqk_sb[:], wqk.rearrange("(o p) n -> p o n", p=P))
            wv_sb = pa.tile([P, 8, DL], F32R)
            nc.gpsimd.dma_start(wv_sb[:], wv.rearrange("(o p) n -> p o n", p=P))
            cos_sb = pa.tile([P, T], F32)
            nc.sync.dma_start(cos_sb[:], cos2)
            sin_sb = pa.tile([P, T], F32)
            nc.sync.dma_start(sin_sb[:], sin2)
            bias_sb = pa.tile([P, 8 + DL], F32)
            nc.sync.dma_start(bias_sb[:], bias)
            perm_sb = pa.tile([P, P], F32R)
            nc.gpsimd.dma_start(perm_sb[:], perm)

            for tcn in range(NTC):
                ts0 = tcn * TCH
                xT = pa.tile([P, 8, TCH], F32R, tag="xT")
                for i in range(TCH // P):
                    stg = pstg.tile([P, C], F32, tag="stg")
                    nc.sync.dma_start(stg[:], xb[ts0 + i * P: ts0 + (i + 1) * P, :])
                    for quad in range(2):
                        pst = psA.tile([P, 512], F32)
                        for q in range(4):
                            cc = quad * 4 + q
                            nc.tensor.matmul(
                                pst[:, q * P:(q + 1) * P],
                                stg[:, cc * P:(cc + 1) * P], ident[:],
                                is_transpose=True, skip_group_check=True)
                        if quad % 2:
                            nc.scalar.copy(
                                xT[:, quad * 4:(quad + 1) * 4, i * P:(i + 1) * P],
                                pst.rearrange("p (a b) -> p a b", b=P))
                        else:
                            nc.vector.tensor_copy(
                                xT[:, quad * 4:(quad + 1) * 4, i * P:(i + 1) * P],
                                pst.rearrange("p (a b) -> p a b", b=P))
                for j in range(8):
                    psq = psQ.tile([P, TCH], F32)
                    for cc in range(8):
                        nc.tensor.matmul(
                            psq[:],
                            wqk_sb[:, cc, j * P:(j + 1) * P],
                            xT[:, cc, :],
                            start=(cc == 0), stop=(cc == 7))
                    t1 = ptmp.tile([P, TCH], F32R, tag="t1")
                    nc.vector.tensor_scalar_add(t1[:], psq[:], bias_sb[:, j:j + 1])
                    psw = psW.tile([P, TCH], F32)
                    nc.tensor.matmul(psw[:], perm_sb[:], t1[:],
                                     start=True, stop=True)
                    dst = qkT[:, j, ts0:ts0 + TCH]
                    nc.vector.tensor_mul(dst, t1[:], cos_sb[:, ts0:ts0 + TCH])
                    swp = ptmp.tile([P, TCH], F32, tag="swp")
                    nc.vector.tensor_mul(swp[:], psw[:], sin_sb[:, ts0:ts0 + TCH])
                    nc.gpsimd.tensor_tensor(dst, dst, swp[:], mybir.AluOpType.add)
                for i in range(TCH // P):
                    ti = tcn * (TCH // P) + i
                    psv = psV.tile([P, DL], F32)
                    for cc in range(8):
                        nc.tensor.matmul(
                            psv[:],
                            xT[:, cc, i * P:(i + 1) * P],
                            wv_sb[:, cc, :],
                            start=(cc == 0), stop=(cc == 7))
                    vv = vsb[:, ti].rearrange("p (h e) -> p h e", e=65)
                    nc.vector.tensor_tensor(
                        vv[:, :, 0:64],
                        psv.rearrange("p (h e) -> p h e", e=64),
                        bias_sb[:, 8:8 + DL].rearrange("p (h e) -> p h e", e=64),
                        mybir.AluOpType.add)
                    nc.vector.memset(vv[:, :, 64:65], 1.0)

        # ---------------- Phase B: attention ----------------
        if "b" not in phases:
            return
        with tc.tile_pool(name="phb", bufs=1) as pb:
            mask_sb = pb.tile([P, 4, 512], BF16)
            nc.sync.dma_start(mask_sb[:], mask)
            yT = pb.tile([P, 4, T], F32R)
            wp_sb = pb.tile([P, 4, C], F32R)
            nc.gpsimd.dma_start(wp_sb[:], wp.rearrange("(o p) n -> p o n", p=P))

            ones_sb = pb.tile([P, 64], BF16)
            nc.vector.memset(ones_sb[:], 1.0)
            with tc.tile_pool(name="pT", bufs=8) as ppt, \
                 tc.tile_pool(name="rcp", bufs=2) as prcp, \
                 tc.tile_pool(name="psS", bufs=2, space="PSUM") as psS, \
                 tc.tile_pool(name="psO", bufs=2, space="PSUM") as psO, \
                 tc.tile_pool(name="psD", bufs=2, space="PSUM") as psDp:
                vg = vsb.rearrange("p a (h e) -> p a h e", e=65)
                for qc in range(4):
                    for g in range(4):
                        nkc = 4 * qc + 4
                        psot = psO.tile([P, 512], F32, tag="psO")
                        psd = psDp.tile([P, 512], F32, tag="psD")
                        for kc in range(nkc):
                            pss = psS.tile([P, 1024], F32, tag="psS")
                            for hh in range(2):
                                pb0 = hh * 64
                                nc.tensor.matmul(
                                    pss[:, hh * 512:(hh + 1) * 512],
                                    qkT[pb0:pb0 + 64, 4 + g, kc * P:(kc + 1) * P],
                                    qkT[pb0:pb0 + 64, g, qc * 512:(qc + 1) * 512],
                                    start=True, stop=True)
                            pt = ppt.tile([P, 1024], BF16, tag="pt")
                            nc.scalar.activation(pt[:], pss[:], Exp)
                            if kc >= 4 * qc:
                                m = kc - 4 * qc
                                ptv = pt.rearrange("p (a b) -> p a b", b=512)
                                nc.vector.tensor_mul(
                                    ptv, ptv,
                                    mask_sb[:, m:m + 1, :].to_broadcast(
                                        (P, 2, 512)))
                            for hh in range(2):
                                nc.tensor.matmul(
                                    psot[hh * 64:(hh + 1) * 64, :],
                                    vg[:, kc, 2 * g + hh][:, 0:64],
                                    pt[:, hh * 512:(hh + 1) * 512],
                                    start=(kc == 0), stop=(kc == nkc - 1),
                                    tile_position=(0, hh * 64),
                                    skip_group_check=True)
                                nc.tensor.matmul(
                                    psd[hh * 64:(hh + 1) * 64, :],
                                    ones_sb[:],
                                    pt[:, hh * 512:(hh + 1) * 512],
                                    start=(kc == 0), stop=(kc == nkc - 1),
                                    tile_position=(0, hh * 64),
                                    skip_group_check=True)
                        rcp = prcp.tile([P, 512], F32, tag="rcp")
                        nc.vector.reciprocal(rcp[:], psd[:])
                        nc.vector.tensor_mul(
                            yT[:, g, qc * 512:(qc + 1) * 512], psot[:], rcp[:])

            # ---------------- Phase C: output projection ----------------
            if "c" not in phases:
                return
            with tc.tile_pool(name="ost", bufs=3) as post, \
                 tc.tile_pool(name="psP", bufs=4, space="PSUM") as psP:
                for ti in range(16):
                    for n in range(2):
                        psp = psP.tile([P, 512], F32)
                        for g in range(4):
                            nc.tensor.matmul(
                                psp[:],
                                yT[:, g, ti * P:(ti + 1) * P],
                                wp_sb[:, g, n * 512:(n + 1) * 512],
                                start=(g == 0), stop=(g == 3))
                        ost = post.tile([P, 512], F32)
                        nc.scalar.copy(ost[:], psp[:])
                        nc.sync.dma_start(
                            out[ti * P:(ti + 1) * P, n * 512:(n + 1) * 512], ost[:])


def build_nc():
    nc = bacc.Bacc("TRN2", target_bir_lowering=False, debug=False)
    xb = nc.dram_tensor("xb", [T, C], F32, kind="ExternalInput").ap()
    wqk = nc.dram_tensor("wqk", [C, 2 * DL], F32, kind="ExternalInput").ap()
    wv = nc.dram_tensor("wv", [C, DL], F32, kind="ExternalInput").ap()
    wp = nc.dram_tensor("wp", [DL, C], F32, kind="ExternalInput").ap()
    cos2 = nc.dram_tensor("cos2", [P, T], F32, kind="ExternalInput").ap()
    sin2 = nc.dram_tensor("sin2", [P, T], F32, kind="ExternalInput").ap()
    bias = nc.dram_tensor("bias", [P, 8 + DL], F32, kind="ExternalInput").ap()
    mask = nc.dram_tensor("mask", [P, 4, 512], BF16, kind="ExternalInput").ap()
    perm = nc.dram_tensor("perm", [P, P], F32, kind="ExternalInput").ap()
    out = nc.dram_tensor("out", [T, C], F32, kind="ExternalOutput").ap()
    with tile.TileContext(nc) as tc:
        _emit(tc, xb, wqk, wv, wp, cos2, sin2, bias, mask, perm, out)
    nc.compile()
    return nc


def rope_tables():
    inv_freq = 1.0 / (ROPE_BASE ** (np.arange(0, D, 2, dtype=np.float64) / D))
    t = np.arange(T, dtype=np.float64)
    freqs = np.outer(t, inv_freq)                      # [T, 32]
    emb = np.concatenate([freqs, freqs], axis=-1)      # [T, 64]
    cosT = np.cos(emb).T.astype(np.float32)            # [64, T]
    sinT = np.sin(emb).T.astype(np.float32)
    cos2 = np.tile(cosT, (2, 1)).copy()                # [128, T]
    sin2 = np.tile(sinT, (2, 1)).copy()
    return cos2, sin2


def perm_matrix():
    pm = np.zeros((P, P), dtype=np.float32)
    for base in (0, 64):
        for d in range(32):
            pm[base + d + 32, base + d] = -1.0       # rot_half: -x2 into top
            pm[base + d, base + d + 32] = 1.0        # +x1 into bottom
    return pm


def causal_masks():
    k = np.arange(P)[:, None]
    q = np.arange(512)[None, :]
    import ml_dtypes
    m = np.stack([(mm * P + k <= q) for mm in range(4)], axis=1)
    return np.ascontiguousarray(m.astype(ml_dtypes.bfloat16))  # [128, 4, 512]


def host_inputs(x, W_qkv, b_qkv, W_proj, b_proj):
    x = np.asarray(x, dtype=np.float32)
    W_qkv = np.asarray(W_qkv, dtype=np.float32)
    b_qkv = np.asarray(b_qkv, dtype=np.float32)
    W_proj = np.asarray(W_proj, dtype=np.float32)
    scale = 1.0 / math.sqrt(D)
    cos2, sin2 = rope_tables()
    masks = causal_masks()
    pm = perm_matrix()
    in_maps = []
    for core in range(NCORES):
        b = core // 2
        hg = core % 2
        s = hg * DL
        wq = W_qkv[:, s:s + DL] * scale
        wk = W_qkv[:, C + s:C + s + DL]
        wqk = np.ascontiguousarray(np.concatenate([wq, wk], axis=1))
        wv = np.ascontiguousarray(W_qkv[:, 2 * C + s:2 * C + s + DL])
        wp = np.ascontiguousarray(W_proj[s:s + DL, :])
        bq = b_qkv[s:s + DL] * scale
        bk = b_qkv[C + s:C + s + DL]
        bv = b_qkv[2 * C + s:2 * C + s + DL]
        bqk = np.concatenate([bq, bk]).reshape(8, P).T          # [128, 8]
        bvb = np.tile(bv[None, :], (P, 1))                      # [128, 512]
        bias = np.ascontiguousarray(
            np.concatenate([bqk, bvb], axis=1).astype(np.float32))
        in_maps.append({
            "xb": np.ascontiguousarray(x[b]),
            "wqk": wqk, "wv": wv, "wp": wp,
            "cos2": cos2, "sin2": sin2, "bias": bias, "mask": masks,
            "perm": pm,
        })
    return in_maps


_NC_CACHE = {}


def run(in_maps, **kwargs):
    if "nc" not in _NC_CACHE:
        _NC_CACHE["nc"] = build_nc()
    return run_bass_kernel_spmd(
        _NC_CACHE["nc"], in_maps, core_ids=list(range(NCORES)), **kwargs)


def kernel(x, W_qkv, b_qkv, W_proj, b_proj, **extra):
    in_maps = host_inputs(x, W_qkv, b_qkv, W_proj, b_proj)
    res = run(in_maps)
    b_proj = np.asarray(b_proj, dtype=np.float32)
    out = np.empty((B, T, C), dtype=np.float32)
    for b in range(B):
        out[b] = res.results[2 * b]["out"] + res.results[2 * b + 1]["out"] + b_proj
    return out



# revision 21
# speedup vs baseline: 1.0863x; 1.0863x over previous
"""Trainium2 Bass kernel for multi-head causal attention with RoPE.

Problem: x[4,2048,1024] -> MHA(16 heads, head_dim 64, RoPE, causal) -> [4,2048,1024]

Sharding: 8 cores = 4 batches x 2 head-groups (8 heads each, Megatron-style).
Each core computes a partial [T, C] projection output for its batch; the host
sums the two head-group partials per batch and adds b_proj.

Per-core dataflow, chunked by 512-row t-blocks so projection/attention/output
DMA all overlap (chunk qc only attends to k-chunks <= qc, so QKV for chunk qc
is ready exactly when attention chunk qc needs it):
  A(tcn): x^T via PE transposes (bf16), Q^T/K^T in [c', t] layout with RoPE
          fused on the PSUM->SBUF path, V in [t, c'] bf16 with a ones column
          (even heads [V|1], odd heads [1|V])
  B(qc=tcn): per head-pair g: scores S^T = K Q^T (bf16), block-causal with
          partial-width diagonal blocks; exp on ACT (no max subtraction,
          scores O(+-6)); P@V with the 65-col [V|1] stationary so the softmax
          denominator accumulates as a 65th PSUM row for free; denominator
          broadcast via a rank-1 PE matmul; normalize on DVE writing y^T bf16
  C(tcn): y^T @ W_proj (bf16) for this chunk, bf16 partial out DMA'd to HBM

Weights are pre-cast to bf16 and pre-laid-out on the host so every DMA moves
>=2KB contiguous runs at full modeled bandwidth; wqk is j-chunked so the first
QKV matmul can start ~2us in.
"""

import math
import sys

import numpy as np

if "/opt/trn_rl_repo" not in sys.path:
    sys.path.insert(0, "/opt/trn_rl_repo")

import concourse.bass as bass
import concourse.tile as tile
from concourse import bacc
from concourse import mybir
from concourse.bass_utils import run_bass_kernel_spmd
from concourse.masks import make_identity

B, T, C = 4, 2048, 1024
NH, D = 16, 64
HL = 8              # local heads per core
DL = HL * D         # 512
NCORES = 8
P = 128
TCH = 512           # t-chunk width
NTC = T // TCH
ROPE_BASE = 10000.0

F32 = mybir.dt.float32
F32R = mybir.dt.float32r
F16 = mybir.dt.float16
BF16 = mybir.dt.bfloat16
EXP_SHIFT = -6.25   # exp(s + EXP_SHIFT): cancels in softmax, keeps 1/denom
                    # within fp16 normal range for the broadcast matmul
Exp = mybir.ActivationFunctionType.Exp


def _emit(tc, xb, wqk, wv, wp, cos2, sin2, bias, mask, perm, out):
    nc = tc.nc
    with tc.tile_pool(name="pers", bufs=1) as pers:
        qkT = pers.tile([P, 8, T], BF16)          # j 0-3: Q pairs, 4-7: K pairs
        # V per head pair, both stationaries padded to M=128 (dst must span a
        # full legal partition range): even head [V(64)|1|0(63)] puts its
        # softmax denominator at PSUM row 64; odd head [1|0(63)|V(64)] puts
        # its denominator at row 0 and V at rows 64..127, partition-aligned
        # with yT's odd-head half. Pad columns are memset once.
        vsb = pers.tile([P, 16, 4 * 256], BF16)   # [t mod 128, t tile, pair*256+e]
        ident = pers.tile([P, P], BF16)
        make_identity(nc, ident)
        # selector matrices for the denominator broadcast: a full-K=128
        # matmul sel^T @ rcp replicates rcp row 64 (sel0) / row 0 (sel1)
        # across 64 output partitions; other rcp rows hit zeros.
        ebias = pers.tile([P, 1], F32)
        nc.vector.memset(ebias[:], EXP_SHIFT)
        sel0 = pers.tile([P, 64], F16)
        nc.vector.memset(sel0[:], 0.0)
        nc.vector.memset(sel0[64:65, :], 1.0)
        sel1 = pers.tile([P, 64], F16)
        nc.vector.memset(sel1[:], 0.0)
        nc.vector.memset(sel1[0:1, :], 1.0)

        wqk_sb = pers.tile([P, 8, 8, P], BF16)    # [p, j, o, n]
        for j in range(8):
            nc.gpsimd.dma_start(wqk_sb[:, j], wqk[j])
        wv_sb = pers.tile([P, 8, DL], BF16)
        nc.gpsimd.dma_start(wv_sb[:], wv)
        wp_sb = pers.tile([P, 4, C], BF16)
        nc.gpsimd.dma_start(wp_sb[:], wp)
        cos_sb = pers.tile([P, T], BF16)
        nc.scalar.dma_start(cos_sb[:], cos2)
        sin_sb = pers.tile([P, T], F32)
        nc.scalar.dma_start(sin_sb[:], sin2)
        bias_sb = pers.tile([P, 8 + DL], F32)
        nc.scalar.dma_start(bias_sb[:], bias)
        mask_sb = pers.tile([P, 4, 512], BF16)
        nc.scalar.dma_start(mask_sb[:], mask)
        perm_sb = pers.tile([P, P], BF16)
        nc.scalar.dma_start(perm_sb[:], perm)

        with tc.tile_pool(name="stage", bufs=3) as pstg, \
             tc.tile_pool(name="xT", bufs=2) as pxT, \
             tc.tile_pool(name="tmp", bufs=4) as ptmp, \
             tc.tile_pool(name="pt", bufs=4) as ppt, \
             tc.tile_pool(name="rcp", bufs=2) as prcp, \
             tc.tile_pool(name="rb", bufs=2) as prb, \
             tc.tile_pool(name="yT", bufs=2) as pyT, \
             tc.tile_pool(name="ost", bufs=3) as post, \
             tc.tile_pool(name="psA", bufs=2, space="PSUM") as psA, \
             tc.tile_pool(name="psQ", bufs=2, space="PSUM") as psQ, \
             tc.tile_pool(name="psS", bufs=2, space="PSUM") as psS:

            # pre-zero the pt ring: diagonal blocks read (then mask to zero)
            # columns their partial-width exp never wrote, so the ring must
            # start finite. Same for the rcp ring (broadcast matmuls contract
            # its unwritten rows against zeros, but they must be finite).
            for _ in range(4):
                ptz = ppt.tile([P, 1024], BF16, tag="pt")
                nc.vector.memset(ptz[:], 0.0)
            for _ in range(2):
                rcpz = prcp.tile([P, 512], F16, tag="rcp")
                nc.vector.memset(rcpz[:], 0.0)

            bias_v = bias_sb[:, 8:8 + DL].rearrange(
                "p (two pr e) -> p two pr e", two=2, e=64)
            bias_ve = bias_v[:, 0]
            bias_vo = bias_v[:, 1]
            vgv = vsb.rearrange("p a (pr e) -> p a pr e", e=256)
            nc.vector.memset(vgv[:, :, :, 64:65], 1.0)
            nc.vector.memset(vgv[:, :, :, 65:128], 0.0)
            nc.vector.memset(vgv[:, :, :, 128:129], 1.0)
            nc.vector.memset(vgv[:, :, :, 129:192], 0.0)

            for tcn in range(NTC):
                ts0 = tcn * TCH
                # ---------------- A: x^T, Q^T/K^T (+RoPE), V ----------------
                xT = pxT.tile([P, 8, TCH], BF16, tag="xT")
                for i in range(TCH // P):
                    stg = pstg.tile([P, C], BF16, tag="stg")
                    nc.sync.dma_start(stg[:], xb[ts0 + i * P: ts0 + (i + 1) * P, :])
                    for quad in range(2):
                        pst = psA.tile([P, 512], BF16, tag="a")
                        for q in range(4):
                            cc = quad * 4 + q
                            nc.tensor.matmul(
                                pst[:, q * P:(q + 1) * P],
                                stg[:, cc * P:(cc + 1) * P], ident[:],
                                is_transpose=True, skip_group_check=True)
                        dstx = xT[:, quad * 4:(quad + 1) * 4, i * P:(i + 1) * P]
                        src = pst.rearrange("p (a b) -> p a b", b=P)
                        if quad % 2:
                            nc.scalar.copy(dstx, src)
                        else:
                            nc.vector.tensor_copy(dstx, src)
                for j in range(8):
                    psq = psQ.tile([P, TCH], F32, tag="q")
                    for cc in range(8):
                        nc.tensor.matmul(
                            psq[:],
                            wqk_sb[:, j, cc, :],
                            xT[:, cc, :],
                            start=(cc == 0), stop=(cc == 7))
                    t1 = ptmp.tile([P, TCH], BF16, tag="t1")
                    nc.vector.tensor_scalar_add(t1[:], psq[:], bias_sb[:, j:j + 1])
                    psw = psA.tile([P, TCH], F32, tag="a")
                    nc.tensor.matmul(psw[:], perm_sb[:], t1[:],
                                     start=True, stop=True)
                    dst = qkT[:, j, ts0:ts0 + TCH]
                    nc.vector.tensor_mul(dst, t1[:], cos_sb[:, ts0:ts0 + TCH])
                    swp = ptmp.tile([P, TCH], BF16, tag="swp")
                    nc.vector.tensor_mul(swp[:], psw[:], sin_sb[:, ts0:ts0 + TCH])
                    nc.vector.tensor_tensor(dst, dst, swp[:], mybir.AluOpType.add)
                for i in range(TCH // P):
                    ti = tcn * (TCH // P) + i
                    psv = psQ.tile([P, DL], F32, tag="q")
                    for cc in range(8):
                        nc.tensor.matmul(
                            psv[:],
                            xT[:, cc, i * P:(i + 1) * P],
                            wv_sb[:, cc, :],
                            start=(cc == 0), stop=(cc == 7))
                    vv = vsb[:, ti].rearrange("p (pr e) -> p pr e", e=256)
                    psvh = psv.rearrange(
                        "p (two pr e) -> p two pr e", two=2, e=64)
                    nc.vector.tensor_tensor(
                        vv[:, :, 0:64], psvh[:, 0],
                        bias_ve, mybir.AluOpType.add)
                    nc.vector.tensor_tensor(
                        vv[:, :, 192:256], psvh[:, 1],
                        bias_vo, mybir.AluOpType.add)

                # ---------------- B: attention for qc = tcn ----------------
                qc = tcn
                nfull = 4 * qc
                yT = pyT.tile([P, 4, TCH], BF16, tag="yT")
                for g in range(4):
                    psO0 = psA.tile([P, 512], F32, tag="a")
                    psO1 = psA.tile([P, 512], F32, tag="a")
                    for kc in range(nfull + 4):
                        m = kc - nfull  # >= 0: diagonal block band
                        pss = psS.tile([P, 1024], F32, tag="pss")
                        pt = ppt.tile([P, 1024], BF16, tag="pt")
                        for hh in range(2):
                            pb0 = hh * 64
                            q0 = m * P if m > 0 else 0
                            nc.tensor.matmul(
                                pss[:, hh * 512 + q0:(hh + 1) * 512],
                                qkT[pb0:pb0 + 64, 4 + g, kc * P:(kc + 1) * P],
                                qkT[pb0:pb0 + 64, g, qc * 512 + q0:(qc + 1) * 512],
                                start=True, stop=True)
                        if m < 0:
                            nc.scalar.activation(pt[:], pss[:], Exp,
                                                 bias=ebias[:])
                        else:
                            q0 = m * P if m > 0 else 0
                            for hh in range(2):
                                nc.scalar.activation(
                                    pt[:, hh * 512 + q0:(hh + 1) * 512],
                                    pss[:, hh * 512 + q0:(hh + 1) * 512], Exp,
                                    bias=ebias[:])
                            ptv = pt.rearrange("p (a b) -> p a b", b=512)
                            nc.vector.tensor_mul(
                                ptv, ptv,
                                mask_sb[:, m:m + 1, :].to_broadcast((P, 2, 512)))
                        nc.tensor.matmul(
                            psO0[:],
                            vgv[:, kc, g][:, 0:128],
                            pt[:, 0:512],
                            start=(kc == 0), stop=(kc == nfull + 3),
                            skip_group_check=True)
                        nc.tensor.matmul(
                            psO1[:],
                            vgv[:, kc, g][:, 128:256],
                            pt[:, 512:1024],
                            start=(kc == 0), stop=(kc == nfull + 3),
                            skip_group_check=True)
                    rcp = prcp.tile([P, 512], F16, tag="rcp")
                    with nc.allow_low_precision(
                            reason="1/denom fits fp16 after EXP_SHIFT"):
                        nc.vector.reciprocal(rcp[64:65, :], psO0[64:65, :])
                        nc.vector.reciprocal(rcp[0:1, :], psO1[0:1, :])
                    psB = psQ.tile([P, 512], F32, tag="q")
                    nc.tensor.matmul(psB[0:64, :], sel0[:], rcp[:],
                                     start=True, stop=True,
                                     skip_group_check=True)
                    nc.tensor.matmul(psB[64:128, :], sel1[:], rcp[:],
                                     start=True, stop=True,
                                     skip_group_check=True)
                    rb = prb.tile([P, 512], F32, tag="rb")
                    nc.scalar.copy(rb[:], psB[:])
                    nc.vector.tensor_mul(yT[0:64, g, :], psO0[0:64, :],
                                         rb[0:64, :])
                    nc.vector.tensor_mul(yT[64:128, g, :], psO1[64:128, :],
                                         rb[64:128, :])

                # ---------------- C: output projection for this chunk ------
                for i in range(TCH // P):
                    for n in range(2):
                        psp = psQ.tile([P, 512], F32, tag="q")
                        for g in range(4):
                            nc.tensor.matmul(
                                psp[:],
                                yT[:, g, i * P:(i + 1) * P],
                                wp_sb[:, g, n * 512:(n + 1) * 512],
                                start=(g == 0), stop=(g == 3))
                        ost = post.tile([P, 512], BF16, tag="ost")
                        nc.scalar.copy(ost[:], psp[:])
                        nc.sync.dma_start(
                            out[ts0 + i * P: ts0 + (i + 1) * P,
                                n * 512:(n + 1) * 512], ost[:])


def build_nc():
    nc = bacc.Bacc("TRN2", target_bir_lowering=False, debug=False)
    xb = nc.dram_tensor("xb", [T, C], BF16, kind="ExternalInput").ap()
    wqk = nc.dram_tensor("wqk", [8, P, 8, P], BF16, kind="ExternalInput").ap()
    wv = nc.dram_tensor("wv", [P, 8, DL], BF16, kind="ExternalInput").ap()
    wp = nc.dram_tensor("wp", [P, 4, C], BF16, kind="ExternalInput").ap()
    cos2 = nc.dram_tensor("cos2", [P, T], BF16, kind="ExternalInput").ap()
    sin2 = nc.dram_tensor("sin2", [P, T], F32, kind="ExternalInput").ap()
    bias = nc.dram_tensor("bias", [P, 8 + DL], F32, kind="ExternalInput").ap()
    mask = nc.dram_tensor("mask", [P, 4, 512], BF16, kind="ExternalInput").ap()
    perm = nc.dram_tensor("perm", [P, P], BF16, kind="ExternalInput").ap()
    out = nc.dram_tensor("out", [T, C], BF16, kind="ExternalOutput").ap()
    with tile.TileContext(nc) as tc:
        _emit(tc, xb, wqk, wv, wp, cos2, sin2, bias, mask, perm, out)
    nc.compile()
    return nc


def rope_tables():
    inv_freq = 1.0 / (ROPE_BASE ** (np.arange(0, D, 2, dtype=np.float64) / D))
    t = np.arange(T, dtype=np.float64)
    freqs = np.outer(t, inv_freq)                      # [T, 32]
    emb = np.concatenate([freqs, freqs], axis=-1)      # [T, 64]
    cosT = np.cos(emb).T.astype(np.float32)            # [64, T]
    sinT = np.sin(emb).T.astype(np.float32)
    cos2 = np.tile(cosT, (2, 1)).copy()                # [128, T]
    sin2 = np.tile(sinT, (2, 1)).copy()
    return cos2, sin2


def perm_matrix():
    pm = np.zeros((P, P), dtype=np.float32)
    for base in (0, 64):
        for d in range(32):
            pm[base + d + 32, base + d] = -1.0       # rot_half: -x2 into top
            pm[base + d, base + d + 32] = 1.0        # +x1 into bottom
    return pm


def causal_masks():
    k = np.arange(P)[:, None]
    q = np.arange(512)[None, :]
    import ml_dtypes
    m = np.stack([(mm * P + k <= q) for mm in range(4)], axis=1)
    return np.ascontiguousarray(m.astype(ml_dtypes.bfloat16))  # [128, 4, 512]


def host_inputs(x, W_qkv, b_qkv, W_proj, b_proj):
    import ml_dtypes
    bf16 = ml_dtypes.bfloat16
    x = np.asarray(x, dtype=np.float32)
    W_qkv = np.asarray(W_qkv, dtype=np.float32)
    b_qkv = np.asarray(b_qkv, dtype=np.float32)
    W_proj = np.asarray(W_proj, dtype=np.float32)
    scale = 1.0 / math.sqrt(D)
    cos2, sin2 = rope_tables()
    cos2 = cos2.astype(bf16)
    masks = causal_masks()
    pm = perm_matrix().astype(bf16)
    in_maps = []
    for core in range(NCORES):
        b = core // 2
        hg = core % 2
        s = hg * DL
        wq = W_qkv[:, s:s + DL] * scale
        wk = W_qkv[:, C + s:C + s + DL]
        wqk_f = np.concatenate([wq, wk], axis=1)                # [1024, 1024]
        # [o*128+p, j*128+n] -> [j, p, o, n]
        wqk_d = np.ascontiguousarray(
            wqk_f.reshape(8, P, 8, P).transpose(2, 1, 0, 3).astype(bf16))
        ord_eo = [0, 2, 4, 6, 1, 3, 5, 7]
        wv_f = W_qkv[:, 2 * C + s:2 * C + s + DL]               # [1024, 512]
        wv_f = wv_f.reshape(C, 8, 64)[:, ord_eo, :].reshape(C, DL)
        wv_d = np.ascontiguousarray(
            wv_f.reshape(8, P, DL).transpose(1, 0, 2).astype(bf16))
        wp_f = W_proj[s:s + DL, :]                              # [512, 1024]
        wp_d = np.ascontiguousarray(
            wp_f.reshape(4, P, C).transpose(1, 0, 2).astype(bf16))
        bq = b_qkv[s:s + DL] * scale
        bk = b_qkv[C + s:C + s + DL]
        bv = b_qkv[2 * C + s:2 * C + s + DL]
        bv = bv.reshape(8, 64)[ord_eo].reshape(DL)
        bqk = np.concatenate([bq, bk]).reshape(8, P).T          # [128, 8]
        bvb = np.tile(bv[None, :], (P, 1))                      # [128, 512]
        bias = np.ascontiguousarray(
            np.concatenate([bqk, bvb], axis=1).astype(np.float32))
        in_maps.append({
            "xb": np.ascontiguousarray(x[b].astype(bf16)),
            "wqk": wqk_d, "wv": wv_d, "wp": wp_d,
            "cos2": cos2, "sin2": sin2, "bias": bias, "mask": masks,
            "perm": pm,
        })
    return in_maps


_NC_CACHE = {}


def run(in_maps, **kwargs):
    if "nc" not in _NC_CACHE:
        _NC_CACHE["nc"] = build_nc()
    return run_bass_kernel_spmd(
        _NC_CACHE["nc"], in_maps, core_ids=list(range(NCORES)), **kwargs)


def kernel(x, W_qkv, b_qkv, W_proj, b_proj, **extra):
    in_maps = host_inputs(x, W_qkv, b_qkv, W_proj, b_proj)
    res = run(in_maps)
    b_proj = np.asarray(b_proj, dtype=np.float32)
    out = np.empty((B, T, C), dtype=np.float32)
    for b in range(B):
        out[b] = (res.results[2 * b]["out"].astype(np.float32)
                  + res.results[2 * b + 1]["out"].astype(np.float32) + b_proj)
    return out


# revision 23
# speedup vs baseline: 1.0934x; 1.0065x over previous
"""Trainium2 Bass kernel for multi-head causal attention with RoPE.

Problem: x[4,2048,1024] -> MHA(16 heads, head_dim 64, RoPE, causal) -> [4,2048,1024]

Sharding: 8 cores = 4 batches x 2 head-groups (8 heads each, Megatron-style).
Each core computes a partial [T, C] projection output for its batch; the host
sums the two head-group partials per batch and adds b_proj.

Per-core dataflow, chunked by 512-row t-blocks so projection/attention/output
DMA all overlap (chunk qc only attends to k-chunks <= qc, so QKV for chunk qc
is ready exactly when attention chunk qc needs it):
  A(tcn): x^T via PE transposes (bf16), Q^T/K^T in [c', t] layout with RoPE
          fused on the PSUM->SBUF path, V in [t, c'] bf16 with a ones column
          (even heads [V|1], odd heads [1|V])
  B(qc=tcn): per head-pair g: scores S^T = K Q^T (bf16), block-causal with
          partial-width diagonal blocks; exp on ACT (no max subtraction,
          scores O(+-6)); P@V with the 65-col [V|1] stationary so the softmax
          denominator accumulates as a 65th PSUM row for free; denominator
          broadcast via a rank-1 PE matmul; normalize on DVE writing y^T bf16
  C(tcn): y^T @ W_proj (bf16) for this chunk, bf16 partial out DMA'd to HBM

Weights are pre-cast to bf16 and pre-laid-out on the host so every DMA moves
>=2KB contiguous runs at full modeled bandwidth; wqk is j-chunked so the first
QKV matmul can start ~2us in.
"""

import math
import sys

import numpy as np

if "/opt/trn_rl_repo" not in sys.path:
    sys.path.insert(0, "/opt/trn_rl_repo")

import concourse.bass as bass
import concourse.tile as tile
from concourse import bacc
from concourse import mybir
from concourse.bass_utils import run_bass_kernel_spmd
from concourse.masks import make_identity

B, T, C = 4, 2048, 1024
NH, D = 16, 64
HL = 8              # local heads per core
DL = HL * D         # 512
NCORES = 8
P = 128
TCH = 512           # t-chunk width
NTC = T // TCH
ROPE_BASE = 10000.0

F32 = mybir.dt.float32
F32R = mybir.dt.float32r
F16 = mybir.dt.float16
BF16 = mybir.dt.bfloat16
EXP_SHIFT = -6.25   # exp(s + EXP_SHIFT): cancels in softmax, keeps 1/denom
                    # within fp16 normal range for the broadcast matmul
Exp = mybir.ActivationFunctionType.Exp


def _emit(tc, xb, wqk, wv, wp, cos2, sin2, bias, mask, perm, out):
    nc = tc.nc
    with tc.tile_pool(name="pers", bufs=1) as pers:
        qkT = pers.tile([P, 8, T], BF16)          # j 0-3: Q pairs, 4-7: K pairs
        # V per head pair, both stationaries padded to M=128 (dst must span a
        # full legal partition range): even head [V(64)|1|0(63)] puts its
        # softmax denominator at PSUM row 64; odd head [1|0(63)|V(64)] puts
        # its denominator at row 0 and V at rows 64..127, partition-aligned
        # with yT's odd-head half. Pad columns are memset once.
        vsb = pers.tile([P, 16, 4 * 256], BF16)   # [t mod 128, t tile, pair*256+e]
        ident = pers.tile([P, P], BF16)
        make_identity(nc, ident)
        # selector matrices for the denominator broadcast: a full-K=128
        # matmul sel^T @ rcp replicates rcp row 64 (sel0) / row 0 (sel1)
        # across 64 output partitions; other rcp rows hit zeros.
        ebias = pers.tile([P, 1], F32)
        nc.vector.memset(ebias[:], EXP_SHIFT)
        sel0 = pers.tile([P, 64], F16)
        nc.vector.memset(sel0[:], 0.0)
        nc.vector.memset(sel0[64:65, :], 1.0)
        sel1 = pers.tile([P, 64], F16)
        nc.vector.memset(sel1[:], 0.0)
        nc.vector.memset(sel1[0:1, :], 1.0)

        wqk_sb = pers.tile([P, 8, 8, P], BF16)    # [p, j, o, n]
        for j in range(8):
            nc.gpsimd.dma_start(wqk_sb[:, j], wqk[j])
        wv_sb = pers.tile([P, 8, DL], BF16)
        nc.gpsimd.dma_start(wv_sb[:], wv)
        wp_sb = pers.tile([P, 4, C], BF16)
        nc.gpsimd.dma_start(wp_sb[:], wp)
        cos_sb = pers.tile([P, T], BF16)
        nc.scalar.dma_start(cos_sb[:], cos2)
        sin_sb = pers.tile([P, T], F32)
        nc.scalar.dma_start(sin_sb[:], sin2)
        bias_sb = pers.tile([P, 8 + DL], F32)
        nc.scalar.dma_start(bias_sb[:], bias)
        mask_sb = pers.tile([P, 4, 512], BF16)
        nc.scalar.dma_start(mask_sb[:], mask)
        perm_sb = pers.tile([P, P], BF16)
        nc.scalar.dma_start(perm_sb[:], perm)

        with tc.tile_pool(name="stage", bufs=3) as pstg, \
             tc.tile_pool(name="xT", bufs=2) as pxT, \
             tc.tile_pool(name="tmp", bufs=4) as ptmp, \
             tc.tile_pool(name="pt", bufs=4) as ppt, \
             tc.tile_pool(name="rcp", bufs=2) as prcp, \
             tc.tile_pool(name="rb", bufs=2) as prb, \
             tc.tile_pool(name="yT", bufs=2) as pyT, \
             tc.tile_pool(name="ost", bufs=3) as post, \
             tc.tile_pool(name="psA", bufs=2, space="PSUM") as psA, \
             tc.tile_pool(name="psQ", bufs=2, space="PSUM") as psQ, \
             tc.tile_pool(name="psS", bufs=2, space="PSUM") as psS:

            # pre-zero the pt ring: diagonal blocks read (then mask to zero)
            # columns their partial-width exp never wrote, so the ring must
            # start finite. Same for the rcp ring (broadcast matmuls contract
            # its unwritten rows against zeros, but they must be finite).
            for _ in range(4):
                ptz = ppt.tile([P, 1024], BF16, tag="pt")
                nc.vector.memset(ptz[:], 0.0)
            for _ in range(2):
                rcpz = prcp.tile([P, 512], F16, tag="rcp")
                nc.vector.memset(rcpz[:], 0.0)

            bias_v = bias_sb[:, 8:8 + DL].rearrange(
                "p (two pr e) -> p two pr e", two=2, e=64)
            bias_ve = bias_v[:, 0]
            bias_vo = bias_v[:, 1]
            vgv = vsb.rearrange("p a (pr e) -> p a pr e", e=256)
            nc.vector.memset(vgv[:, :, :, 64:65], 1.0)
            nc.vector.memset(vgv[:, :, :, 65:128], 0.0)
            nc.vector.memset(vgv[:, :, :, 128:129], 1.0)
            nc.vector.memset(vgv[:, :, :, 129:192], 0.0)

            for tcn in range(NTC):
                ts0 = tcn * TCH
                # ---------------- A: x^T, Q^T/K^T (+RoPE), V ----------------
                xT = pxT.tile([P, 8, TCH], BF16, tag="xT")
                for i in range(TCH // P):
                    stg = pstg.tile([P, C], BF16, tag="stg")
                    nc.sync.dma_start(stg[:], xb[ts0 + i * P: ts0 + (i + 1) * P, :])
                    for quad in range(2):
                        pst = psA.tile([P, 512], BF16, tag="a")
                        for q in range(4):
                            cc = quad * 4 + q
                            nc.tensor.matmul(
                                pst[:, q * P:(q + 1) * P],
                                stg[:, cc * P:(cc + 1) * P], ident[:],
                                is_transpose=True, skip_group_check=True)
                        dstx = xT[:, quad * 4:(quad + 1) * 4, i * P:(i + 1) * P]
                        src = pst.rearrange("p (a b) -> p a b", b=P)
                        if quad % 2:
                            nc.scalar.copy(dstx, src)
                        else:
                            nc.vector.tensor_copy(dstx, src)
                # QK+RoPE, software-pipelined: the perm matmul and RoPE muls
                # for slot j are emitted after slot j+1's GEMM so the PE never
                # stalls on the DVE bias-add.
                rope_prev = None
                for j in range(8):
                    psq = psQ.tile([P, TCH], F32, tag="q")
                    for cc in range(8):
                        nc.tensor.matmul(
                            psq[:],
                            wqk_sb[:, j, cc, :],
                            xT[:, cc, :],
                            start=(cc == 0), stop=(cc == 7))
                    t1 = ptmp.tile([P, TCH], BF16, tag="t1")
                    nc.vector.tensor_scalar_add(t1[:], psq[:], bias_sb[:, j:j + 1])
                    if rope_prev is not None:
                        rope_prev()

                    def rope_now(j=j, t1=t1):
                        psw = psA.tile([P, TCH], F32, tag="a")
                        nc.tensor.matmul(psw[:], perm_sb[:], t1[:],
                                         start=True, stop=True)
                        dst = qkT[:, j, ts0:ts0 + TCH]
                        nc.vector.tensor_mul(dst, t1[:], cos_sb[:, ts0:ts0 + TCH])
                        swp = ptmp.tile([P, TCH], BF16, tag="swp")
                        nc.vector.tensor_mul(swp[:], psw[:],
                                             sin_sb[:, ts0:ts0 + TCH])
                        nc.vector.tensor_tensor(dst, dst, swp[:],
                                                mybir.AluOpType.add)
                    rope_prev = rope_now
                rope_prev()
                for i in range(TCH // P):
                    ti = tcn * (TCH // P) + i
                    psv = psQ.tile([P, DL], F32, tag="q")
                    for cc in range(8):
                        nc.tensor.matmul(
                            psv[:],
                            xT[:, cc, i * P:(i + 1) * P],
                            wv_sb[:, cc, :],
                            start=(cc == 0), stop=(cc == 7))
                    vv = vsb[:, ti].rearrange("p (pr e) -> p pr e", e=256)
                    psvh = psv.rearrange(
                        "p (two pr e) -> p two pr e", two=2, e=64)
                    nc.vector.tensor_tensor(
                        vv[:, :, 0:64], psvh[:, 0],
                        bias_ve, mybir.AluOpType.add)
                    nc.vector.tensor_tensor(
                        vv[:, :, 192:256], psvh[:, 1],
                        bias_vo, mybir.AluOpType.add)

                # ---------------- B: attention for qc = tcn ----------------
                # Software-pipelined: each PV pair is emitted after the NEXT
                # block's scores (so the PE runs scores while the ACT exps the
                # previous block), and each head-pair's normalize is deferred
                # into the next pair's score phase (reciprocal right after the
                # last PV, broadcast/mul two score-blocks later).
                qc = tcn
                nfull = 4 * qc
                yT = pyT.tile([P, 4, TCH], BF16, tag="yT")
                norm_prev = None
                for g in range(4):
                    psO0 = psA.tile([P, 512], F32, tag="a")
                    psO1 = psA.tile([P, 512], F32, tag="a")
                    pv_prev = None
                    for kc in range(nfull + 4):
                        m = kc - nfull  # >= 0: diagonal block band
                        pss = psS.tile([P, 1024], F32, tag="pss")
                        pt = ppt.tile([P, 1024], BF16, tag="pt")
                        for hh in range(2):
                            pb0 = hh * 64
                            q0 = m * P if m > 0 else 0
                            nc.tensor.matmul(
                                pss[:, hh * 512 + q0:(hh + 1) * 512],
                                qkT[pb0:pb0 + 64, 4 + g, kc * P:(kc + 1) * P],
                                qkT[pb0:pb0 + 64, g, qc * 512 + q0:(qc + 1) * 512],
                                start=True, stop=True)
                        if m < 0:
                            nc.scalar.activation(pt[:], pss[:], Exp,
                                                 bias=ebias[:])
                        else:
                            q0 = m * P if m > 0 else 0
                            for hh in range(2):
                                nc.scalar.activation(
                                    pt[:, hh * 512 + q0:(hh + 1) * 512],
                                    pss[:, hh * 512 + q0:(hh + 1) * 512], Exp,
                                    bias=ebias[:])
                            ptv = pt.rearrange("p (a b) -> p a b", b=512)
                            nc.vector.tensor_mul(
                                ptv, ptv,
                                mask_sb[:, m:m + 1, :].to_broadcast((P, 2, 512)))
                        if pv_prev is not None:
                            pv_prev()
                        if kc == 1 and norm_prev is not None:
                            norm_prev()
                            norm_prev = None

                        def pv_now(kc=kc, pt=pt, psO0=psO0, psO1=psO1):
                            nc.tensor.matmul(
                                psO0[:],
                                vgv[:, kc, g][:, 0:128],
                                pt[:, 0:512],
                                start=(kc == 0), stop=(kc == nfull + 3),
                                skip_group_check=True)
                            nc.tensor.matmul(
                                psO1[:],
                                vgv[:, kc, g][:, 128:256],
                                pt[:, 512:1024],
                                start=(kc == 0), stop=(kc == nfull + 3),
                                skip_group_check=True)
                        pv_prev = pv_now
                    pv_prev()
                    rcp = prcp.tile([P, 512], F16, tag="rcp")
                    with nc.allow_low_precision(
                            reason="1/denom fits fp16 after EXP_SHIFT"):
                        nc.vector.reciprocal(rcp[64:65, :], psO0[64:65, :])
                        nc.vector.reciprocal(rcp[0:1, :], psO1[0:1, :])

                    def norm_now(g=g, rcp=rcp, psO0=psO0, psO1=psO1, yT=yT):
                        psB = psQ.tile([P, 512], F32, tag="q")
                        nc.tensor.matmul(psB[0:64, :], sel0[:], rcp[:],
                                         start=True, stop=True,
                                         skip_group_check=True)
                        nc.tensor.matmul(psB[64:128, :], sel1[:], rcp[:],
                                         start=True, stop=True,
                                         skip_group_check=True)
                        rb = prb.tile([P, 512], F32, tag="rb")
                        nc.scalar.copy(rb[:], psB[:])
                        nc.vector.tensor_mul(yT[0:64, g, :], psO0[0:64, :],
                                             rb[0:64, :])
                        nc.vector.tensor_mul(yT[64:128, g, :], psO1[64:128, :],
                                             rb[64:128, :])
                    norm_prev = norm_now
                norm_prev()

                # ---------------- C: output projection for this chunk ------
                for i in range(TCH // P):
                    for n in range(2):
                        psp = psQ.tile([P, 512], F32, tag="q")
                        for g in range(4):
                            nc.tensor.matmul(
                                psp[:],
                                yT[:, g, i * P:(i + 1) * P],
                                wp_sb[:, g, n * 512:(n + 1) * 512],
                                start=(g == 0), stop=(g == 3))
                        ost = post.tile([P, 512], BF16, tag="ost")
                        nc.scalar.copy(ost[:], psp[:])
                        nc.sync.dma_start(
                            out[ts0 + i * P: ts0 + (i + 1) * P,
                                n * 512:(n + 1) * 512], ost[:])


def build_nc():
    nc = bacc.Bacc("TRN2", target_bir_lowering=False, debug=False)
    xb = nc.dram_tensor("xb", [T, C], BF16, kind="ExternalInput").ap()
    wqk = nc.dram_tensor("wqk", [8, P, 8, P], BF16, kind="ExternalInput").ap()
    wv = nc.dram_tensor("wv", [P, 8, DL], BF16, kind="ExternalInput").ap()
    wp = nc.dram_tensor("wp", [P, 4, C], BF16, kind="ExternalInput").ap()
    cos2 = nc.dram_tensor("cos2", [P, T], BF16, kind="ExternalInput").ap()
    sin2 = nc.dram_tensor("sin2", [P, T], F32, kind="ExternalInput").ap()
    bias = nc.dram_tensor("bias", [P, 8 + DL], F32, kind="ExternalInput").ap()
    mask = nc.dram_tensor("mask", [P, 4, 512], BF16, kind="ExternalInput").ap()
    perm = nc.dram_tensor("perm", [P, P], BF16, kind="ExternalInput").ap()
    out = nc.dram_tensor("out", [T, C], BF16, kind="ExternalOutput").ap()
    with tile.TileContext(nc) as tc:
        _emit(tc, xb, wqk, wv, wp, cos2, sin2, bias, mask, perm, out)
    nc.compile()
    return nc


def rope_tables():
    inv_freq = 1.0 / (ROPE_BASE ** (np.arange(0, D, 2, dtype=np.float64) / D))
    t = np.arange(T, dtype=np.float64)
    freqs = np.outer(t, inv_freq)                      # [T, 32]
    emb = np.concatenate([freqs, freqs], axis=-1)      # [T, 64]
    cosT = np.cos(emb).T.astype(np.float32)            # [64, T]
    sinT = np.sin(emb).T.astype(np.float32)
    cos2 = np.tile(cosT, (2, 1)).copy()                # [128, T]
    sin2 = np.tile(sinT, (2, 1)).copy()
    return cos2, sin2


def perm_matrix():
    pm = np.zeros((P, P), dtype=np.float32)
    for base in (0, 64):
        for d in range(32):
            pm[base + d + 32, base + d] = -1.0       # rot_half: -x2 into top
            pm[base + d, base + d + 32] = 1.0        # +x1 into bottom
    return pm


def causal_masks():
    k = np.arange(P)[:, None]
    q = np.arange(512)[None, :]
    import ml_dtypes
    m = np.stack([(mm * P + k <= q) for mm in range(4)], axis=1)
    return np.ascontiguousarray(m.astype(ml_dtypes.bfloat16))  # [128, 4, 512]


def host_inputs(x, W_qkv, b_qkv, W_proj, b_proj):
    import ml_dtypes
    bf16 = ml_dtypes.bfloat16
    x = np.asarray(x, dtype=np.float32)
    W_qkv = np.asarray(W_qkv, dtype=np.float32)
    b_qkv = np.asarray(b_qkv, dtype=np.float32)
    W_proj = np.asarray(W_proj, dtype=np.float32)
    scale = 1.0 / math.sqrt(D)
    cos2, sin2 = rope_tables()
    cos2 = cos2.astype(bf16)
    masks = causal_masks()
    pm = perm_matrix().astype(bf16)
    in_maps = []
    for core in range(NCORES):
        b = core // 2
        hg = core % 2
        s = hg * DL
        wq = W_qkv[:, s:s + DL] * scale
        wk = W_qkv[:, C + s:C + s + DL]
        wqk_f = np.concatenate([wq, wk], axis=1)                # [1024, 1024]
        # [o*128+p, j*128+n] -> [j, p, o, n]
        wqk_d = np.ascontiguousarray(
            wqk_f.reshape(8, P, 8, P).transpose(2, 1, 0, 3).astype(bf16))
        ord_eo = [0, 2, 4, 6, 1, 3, 5, 7]
        wv_f = W_qkv[:, 2 * C + s:2 * C + s + DL]               # [1024, 512]
        wv_f = wv_f.reshape(C, 8, 64)[:, ord_eo, :].reshape(C, DL)
        wv_d = np.ascontiguousarray(
            wv_f.reshape(8, P, DL).transpose(1, 0, 2).astype(bf16))
        wp_f = W_proj[s:s + DL, :]                              # [512, 1024]
        wp_d = np.ascontiguousarray(
            wp_f.reshape(4, P, C).transpose(1, 0, 2).astype(bf16))
        bq = b_qkv[s:s + DL] * scale
        bk = b_qkv[C + s:C + s + DL]
        bv = b_qkv[2 * C + s:2 * C + s + DL]
        bv = bv.reshape(8, 64)[ord_eo].reshape(DL)
        bqk = np.concatenate([bq, bk]).reshape(8, P).T          # [128, 8]
        bvb = np.tile(bv[None, :], (P, 1))                      # [128, 512]
        bias = np.ascontiguousarray(
            np.concatenate([bqk, bvb], axis=1).astype(np.float32))
        in_maps.append({
            "xb": np.ascontiguousarray(x[b].astype(bf16)),
            "wqk": wqk_d, "wv": wv_d, "wp": wp_d,
            "cos2": cos2, "sin2": sin2, "bias": bias, "mask": masks,
            "perm": pm,
        })
    return in_maps


_NC_CACHE = {}


def run(in_maps, **kwargs):
    if "nc" not in _NC_CACHE:
        _NC_CACHE["nc"] = build_nc()
    return run_bass_kernel_spmd(
        _NC_CACHE["nc"], in_maps, core_ids=list(range(NCORES)), **kwargs)


def kernel(x, W_qkv, b_qkv, W_proj, b_proj, **extra):
    in_maps = host_inputs(x, W_qkv, b_qkv, W_proj, b_proj)
    res = run(in_maps)
    b_proj = np.asarray(b_proj, dtype=np.float32)
    out = np.empty((B, T, C), dtype=np.float32)
    for b in range(B):
        out[b] = (res.results[2 * b]["out"].astype(np.float32)
                  + res.results[2 * b + 1]["out"].astype(np.float32) + b_proj)
    return out


# revision 26
# speedup vs baseline: 1.2189x; 1.1149x over previous
"""Trainium2 Bass kernel for multi-head causal attention with RoPE.

Problem: x[4,2048,1024] -> MHA(16 heads, head_dim 64, RoPE, causal) -> [4,2048,1024]

Sharding: 8 cores = 4 batches x 2 head-groups (8 heads each, Megatron-style).
Each core computes a partial [T, C] projection output for its batch; the host
sums the two head-group partials per batch and adds b_proj.

Per-core dataflow, chunked by 512-row t-blocks so projection/attention/output
DMA all overlap (chunk qc only attends to k-chunks <= qc, so QKV for chunk qc
is ready exactly when attention chunk qc needs it):
  A(tcn): x^T via PE transposes (bf16), Q^T/K^T in [c', t] layout with RoPE
          fused on the PSUM->SBUF path, V in [t, c'] bf16 with a ones column
          (even heads [V|1], odd heads [1|V])
  B(qc=tcn): per head-pair g: scores S^T = K Q^T (bf16), block-causal with
          partial-width diagonal blocks; exp on ACT (no max subtraction,
          scores O(+-6)); P@V with the 65-col [V|1] stationary so the softmax
          denominator accumulates as a 65th PSUM row for free; denominator
          broadcast via a rank-1 PE matmul; normalize on DVE writing y^T bf16
  C(tcn): y^T @ W_proj (bf16) for this chunk, bf16 partial out DMA'd to HBM

Weights are pre-cast to bf16 and pre-laid-out on the host so every DMA moves
>=2KB contiguous runs at full modeled bandwidth; wqk is j-chunked so the first
QKV matmul can start ~2us in.
"""

import math
import sys

import numpy as np

if "/opt/trn_rl_repo" not in sys.path:
    sys.path.insert(0, "/opt/trn_rl_repo")

import concourse.bass as bass
import concourse.tile as tile
from concourse import bacc
from concourse import mybir
from concourse.bass_utils import run_bass_kernel_spmd
from concourse.masks import make_identity

B, T, C = 4, 2048, 1024
NH, D = 16, 64
HL = 8              # local heads per core
DL = HL * D         # 512
NCORES = 8
P = 128
TCH = 512           # t-chunk width
NTC = T // TCH
ROPE_BASE = 10000.0

F32 = mybir.dt.float32
F32R = mybir.dt.float32r
F16 = mybir.dt.float16
BF16 = mybir.dt.bfloat16
EXP_SHIFT = -6.25   # exp(s + EXP_SHIFT): cancels in softmax, keeps 1/denom
                    # within fp16 normal range for the broadcast matmul
Exp = mybir.ActivationFunctionType.Exp


def _emit(tc, xb, wqk, wv, wp, cos2, sin2, bias, mask, perm, out):
    nc = tc.nc
    with tc.tile_pool(name="pers", bufs=1) as pers:
        qkT = pers.tile([P, 8, T], BF16)          # j 0-3: Q pairs, 4-7: K pairs
        # V per head pair, both stationaries padded to M=128 (dst must span a
        # full legal partition range): even head [V(64)|1|0(63)] puts its
        # softmax denominator at PSUM row 64; odd head [1|0(63)|V(64)] puts
        # its denominator at row 0 and V at rows 64..127, partition-aligned
        # with yT's odd-head half. Pad columns are memset once.
        vsb = pers.tile([P, 16, 4 * 256], BF16)   # [t mod 128, t tile, pair*256+e]
        ident = pers.tile([P, P], BF16)
        make_identity(nc, ident)
        # selector matrices for the denominator broadcast: a full-K=128
        # matmul sel^T @ rcp replicates rcp row 64 (sel0) / row 0 (sel1)
        # across 64 output partitions; other rcp rows hit zeros.
        ebias = pers.tile([P, 1], F32)
        nc.vector.memset(ebias[:], EXP_SHIFT)
        sel0 = pers.tile([P, 64], F16)
        nc.vector.memset(sel0[:], 0.0)
        nc.vector.memset(sel0[64:65, :], 1.0)
        sel1 = pers.tile([P, 64], F16)
        nc.vector.memset(sel1[:], 0.0)
        nc.vector.memset(sel1[0:1, :], 1.0)

        wqk_sb = pers.tile([P, 8, 8, P], BF16)    # [p, j, o, n]
        wv_sb = pers.tile([P, 8, DL], BF16)
        wp_sb = pers.tile([P, 4, C], BF16)
        cos_sb = pers.tile([P, T], BF16)
        sin_sb = pers.tile([P, T], F32)
        bias_sb = pers.tile([P, 8 + DL], F32)
        mask_sb = pers.tile([P, 4, 512], BF16)
        perm_sb = pers.tile([P, P], BF16)
        # all weight loads on the ACT HWDGE queue (cheap descriptor gen),
        # ordered by first use so the PE can start ~2us in
        for j in range(4):
            nc.scalar.dma_start(wqk_sb[:, j], wqk[j])
        nc.scalar.dma_start(perm_sb[:], perm)
        nc.scalar.dma_start(cos_sb[:], cos2)
        nc.scalar.dma_start(sin_sb[:], sin2)
        nc.scalar.dma_start(bias_sb[:], bias)
        for j in range(4, 8):
            nc.scalar.dma_start(wqk_sb[:, j], wqk[j])
        nc.scalar.dma_start(wv_sb[:], wv)
        nc.scalar.dma_start(mask_sb[:], mask)
        nc.scalar.dma_start(wp_sb[:], wp)

        with tc.tile_pool(name="stage", bufs=3) as pstg, \
             tc.tile_pool(name="xT", bufs=2) as pxT, \
             tc.tile_pool(name="tmp", bufs=4) as ptmp, \
             tc.tile_pool(name="pt", bufs=6) as ppt, \
             tc.tile_pool(name="rcp", bufs=2) as prcp, \
             tc.tile_pool(name="rb", bufs=2) as prb, \
             tc.tile_pool(name="yT", bufs=2) as pyT, \
             tc.tile_pool(name="ost", bufs=3) as post, \
             tc.tile_pool(name="psA", bufs=2, space="PSUM") as psA, \
             tc.tile_pool(name="psQ", bufs=2, space="PSUM") as psQ, \
             tc.tile_pool(name="psS", bufs=2, space="PSUM") as psS:

            # pre-zero the pt ring: diagonal blocks read (then mask to zero)
            # columns their partial-width exp never wrote, so the ring must
            # start finite. Same for the rcp ring (broadcast matmuls contract
            # its unwritten rows against zeros, but they must be finite).
            for _ in range(6):
                ptz = ppt.tile([P, 1024], BF16, tag="pt")
                nc.gpsimd.memset(ptz[:], 0.0)
            for _ in range(2):
                rcpz = prcp.tile([P, 512], F16, tag="rcp")
                nc.gpsimd.memset(rcpz[:], 0.0)

            bias_v = bias_sb[:, 8:8 + DL].rearrange(
                "p (two pr e) -> p two pr e", two=2, e=64)
            bias_ve = bias_v[:, 0]
            bias_vo = bias_v[:, 1]
            vgv = vsb.rearrange("p a (pr e) -> p a pr e", e=256)
            nc.gpsimd.memset(vgv[:, :, :, 64:65], 1.0)
            nc.gpsimd.memset(vgv[:, :, :, 65:128], 0.0)
            nc.gpsimd.memset(vgv[:, :, :, 128:129], 1.0)
            nc.gpsimd.memset(vgv[:, :, :, 129:192], 0.0)

            def a_units(tcn):
                """Emitter units for chunk tcn's QKV phase (transposes, QK
                GEMM+RoPE, V GEMM), interleavable into the previous chunk's
                attention phase."""
                ts0 = tcn * TCH
                xT = pxT.tile([P, 8, TCH], BF16, tag="xT")
                units = []

                def tr_unit(i, xT=xT, ts0=ts0):
                    stg = pstg.tile([P, C], BF16, tag="stg")
                    nc.sync.dma_start(stg[:],
                                      xb[ts0 + i * P: ts0 + (i + 1) * P, :])
                    for quad in range(2):
                        pst = psA.tile([P, 512], BF16, tag="a")
                        for q in range(4):
                            cc = quad * 4 + q
                            nc.tensor.matmul(
                                pst[:, q * P:(q + 1) * P],
                                stg[:, cc * P:(cc + 1) * P], ident[:],
                                is_transpose=True, skip_group_check=True)
                        dstx = xT[:, quad * 4:(quad + 1) * 4, i * P:(i + 1) * P]
                        src = pst.rearrange("p (a b) -> p a b", b=P)
                        if quad % 2:
                            nc.scalar.copy(dstx, src)
                        else:
                            nc.vector.tensor_copy(dstx, src)
                for i in range(TCH // P):
                    units.append((False, lambda i=i: tr_unit(i)))

                rope_st = {"prev": None}

                def qk_unit(j, last, xT=xT, ts0=ts0):
                    psq = psQ.tile([P, TCH], F32, tag="q")
                    for cc in range(8):
                        nc.tensor.matmul(
                            psq[:],
                            wqk_sb[:, j, cc, :],
                            xT[:, cc, :],
                            start=(cc == 0), stop=(cc == 7))
                    t1 = ptmp.tile([P, TCH], BF16, tag="t1")
                    nc.vector.tensor_scalar_add(t1[:], psq[:],
                                                bias_sb[:, j:j + 1])
                    if rope_st["prev"] is not None:
                        rope_st["prev"]()

                    def rope_now(j=j, t1=t1):
                        psw = psA.tile([P, TCH], F32, tag="a")
                        nc.tensor.matmul(psw[:], perm_sb[:], t1[:],
                                         start=True, stop=True)
                        dst = qkT[:, j, ts0:ts0 + TCH]
                        nc.vector.tensor_mul(dst, t1[:],
                                             cos_sb[:, ts0:ts0 + TCH])
                        swp = ptmp.tile([P, TCH], BF16, tag="swp")
                        nc.vector.tensor_mul(swp[:], psw[:],
                                             sin_sb[:, ts0:ts0 + TCH])
                        nc.vector.tensor_tensor(dst, dst, swp[:],
                                                mybir.AluOpType.add)
                    rope_st["prev"] = rope_now
                    if last:
                        rope_st["prev"]()
                        rope_st["prev"] = None
                for j in range(8):
                    units.append((False, lambda j=j: qk_unit(j, j == 7)))

                def v_unit(i, xT=xT, tcn=tcn):
                    ti = tcn * (TCH // P) + i
                    psv = psQ.tile([P, DL], F32, tag="q")
                    for cc in range(8):
                        nc.tensor.matmul(
                            psv[:],
                            xT[:, cc, i * P:(i + 1) * P],
                            wv_sb[:, cc, :],
                            start=(cc == 0), stop=(cc == 7))
                    vv = vsb[:, ti].rearrange("p (pr e) -> p pr e", e=256)
                    psvh = psv.rearrange(
                        "p (two pr e) -> p two pr e", two=2, e=64)
                    nc.vector.tensor_tensor(
                        vv[:, :, 0:64], psvh[:, 0],
                        bias_ve, mybir.AluOpType.add)
                    nc.vector.tensor_tensor(
                        vv[:, :, 192:256], psvh[:, 1],
                        bias_vo, mybir.AluOpType.add)
                for i in range(TCH // P):
                    units.append((True, lambda i=i: v_unit(i)))
                return units

            def c_units(tcn, yT):
                """Projection units for chunk tcn; require yT fully normalized.
                Emitted inside the NEXT chunk's attention phase."""
                ts0 = tcn * TCH
                units = []

                def c_unit(i, n, yT=yT, ts0=ts0):
                    psp = psQ.tile([P, 512], F32, tag="q")
                    for g in range(4):
                        nc.tensor.matmul(
                            psp[:],
                            yT[:, g, i * P:(i + 1) * P],
                            wp_sb[:, g, n * 512:(n + 1) * 512],
                            start=(g == 0), stop=(g == 3))
                    ost = post.tile([P, 512], BF16, tag="ost")
                    nc.scalar.copy(ost[:], psp[:])
                    nc.sync.dma_start(
                        out[ts0 + i * P: ts0 + (i + 1) * P,
                            n * 512:(n + 1) * 512], ost[:])
                for i in range(TCH // P):
                    for n in range(2):
                        units.append((True, lambda i=i, n=n: c_unit(i, n)))
                return units

            def b_phase(qc, feed):
                """Attention for chunk qc. `feed` holds interleavable units
                (next chunk's A, previous chunk's C) emitted at head-pair
                boundaries so the PE chews on them while the ACT exps.
                PV matmuls lag their scores by two blocks."""
                nfull = 4 * qc
                yT = pyT.tile([P, 4, TCH], BF16, tag="yT")
                for g in range(4):
                    psO0 = psA.tile([P, 512], F32, tag="a")
                    psO1 = psA.tile([P, 512], F32, tag="a")
                    pv_q = []
                    for kc in range(nfull + 4):
                        m = kc - nfull  # >= 0: diagonal block band
                        pss = psS.tile([P, 1024], F32, tag="pss")
                        pt = ppt.tile([P, 1024], BF16, tag="pt")
                        for hh in range(2):
                            pb0 = hh * 64
                            q0 = m * P if m > 0 else 0
                            nc.tensor.matmul(
                                pss[:, hh * 512 + q0:(hh + 1) * 512],
                                qkT[pb0:pb0 + 64, 4 + g, kc * P:(kc + 1) * P],
                                qkT[pb0:pb0 + 64, g,
                                    qc * 512 + q0:(qc + 1) * 512],
                                start=True, stop=True)
                        if m < 0:
                            nc.scalar.activation(pt[:], pss[:], Exp,
                                                 bias=ebias[:])
                        else:
                            q0 = m * P if m > 0 else 0
                            for hh in range(2):
                                nc.scalar.activation(
                                    pt[:, hh * 512 + q0:(hh + 1) * 512],
                                    pss[:, hh * 512 + q0:(hh + 1) * 512], Exp,
                                    bias=ebias[:])
                            ptv = pt.rearrange("p (a b) -> p a b", b=512)
                            nc.vector.tensor_mul(
                                ptv, ptv,
                                mask_sb[:, m:m + 1, :].to_broadcast(
                                    (P, 2, 512)))
                        if len(pv_q) >= 2:
                            pv_q.pop(0)()

                        def pv_now(kc=kc, pt=pt, psO0=psO0, psO1=psO1):
                            nc.tensor.matmul(
                                psO0[:],
                                vgv[:, kc, g][:, 0:128],
                                pt[:, 0:512],
                                start=(kc == 0), stop=(kc == nfull + 3),
                                skip_group_check=True)
                            nc.tensor.matmul(
                                psO1[:],
                                vgv[:, kc, g][:, 128:256],
                                pt[:, 512:1024],
                                start=(kc == 0), stop=(kc == nfull + 3),
                                skip_group_check=True)
                        pv_q.append(pv_now)
                    for f in pv_q:
                        f()
                    rcp = prcp.tile([P, 512], F16, tag="rcp")
                    with nc.allow_low_precision(
                            reason="1/denom fits fp16 after EXP_SHIFT"):
                        nc.vector.reciprocal(rcp[64:65, :], psO0[64:65, :])
                        nc.vector.reciprocal(rcp[0:1, :], psO1[0:1, :])
                    # one interleaved unit covers the reciprocal latency.
                    # Only tag-a-free units are legal here: psO is still live,
                    # so a unit allocating from the psA ring would block the
                    # PE ahead of the psB matmuls that free it.
                    if feed and feed[0][0]:
                        feed.pop(0)[1]()
                    psB = psQ.tile([P, 512], F32, tag="q")
                    nc.tensor.matmul(psB[0:64, :], sel0[:], rcp[:],
                                     start=True, stop=True,
                                     skip_group_check=True)
                    nc.tensor.matmul(psB[64:128, :], sel1[:], rcp[:],
                                     start=True, stop=True,
                                     skip_group_check=True)
                    rb = prb.tile([P, 512], F32, tag="rb")
                    nc.vector.tensor_copy(rb[:], psB[:])
                    nc.vector.tensor_mul(yT[0:64, g, :], psO0[0:64, :],
                                         rb[0:64, :])
                    nc.vector.tensor_mul(yT[64:128, g, :], psO1[64:128, :],
                                         rb[64:128, :])
                    # drain a share of the feed at this head-pair boundary
                    share = (len(feed) + 3 - g) // (4 - g) if g < 3 else 0
                    for _ in range(share):
                        feed.pop(0)[1]()
                # whatever's left runs after the last normalize
                while feed:
                    feed.pop(0)[1]()
                return yT

            for _, u in a_units(0):
                u()
            yT_prev = None
            for tcn in range(NTC):
                feed = []
                if yT_prev is not None:
                    feed.extend(c_units(tcn - 1, yT_prev))
                if tcn + 1 < NTC:
                    feed.extend(a_units(tcn + 1))
                yT_prev = b_phase(tcn, feed)
            for _, u in c_units(NTC - 1, yT_prev):
                u()


def build_nc():
    nc = bacc.Bacc("TRN2", target_bir_lowering=False, debug=False)
    xb = nc.dram_tensor("xb", [T, C], BF16, kind="ExternalInput").ap()
    wqk = nc.dram_tensor("wqk", [8, P, 8, P], BF16, kind="ExternalInput").ap()
    wv = nc.dram_tensor("wv", [P, 8, DL], BF16, kind="ExternalInput").ap()
    wp = nc.dram_tensor("wp", [P, 4, C], BF16, kind="ExternalInput").ap()
    cos2 = nc.dram_tensor("cos2", [P, T], BF16, kind="ExternalInput").ap()
    sin2 = nc.dram_tensor("sin2", [P, T], F32, kind="ExternalInput").ap()
    bias = nc.dram_tensor("bias", [P, 8 + DL], F32, kind="ExternalInput").ap()
    mask = nc.dram_tensor("mask", [P, 4, 512], BF16, kind="ExternalInput").ap()
    perm = nc.dram_tensor("perm", [P, P], BF16, kind="ExternalInput").ap()
    out = nc.dram_tensor("out", [T, C], BF16, kind="ExternalOutput").ap()
    with tile.TileContext(nc) as tc:
        _emit(tc, xb, wqk, wv, wp, cos2, sin2, bias, mask, perm, out)
    nc.compile()
    return nc


def rope_tables():
    inv_freq = 1.0 / (ROPE_BASE ** (np.arange(0, D, 2, dtype=np.float64) / D))
    t = np.arange(T, dtype=np.float64)
    freqs = np.outer(t, inv_freq)                      # [T, 32]
    emb = np.concatenate([freqs, freqs], axis=-1)      # [T, 64]
    cosT = np.cos(emb).T.astype(np.float32)            # [64, T]
    sinT = np.sin(emb).T.astype(np.float32)
    cos2 = np.tile(cosT, (2, 1)).copy()                # [128, T]
    sin2 = np.tile(sinT, (2, 1)).copy()
    return cos2, sin2


def perm_matrix():
    pm = np.zeros((P, P), dtype=np.float32)
    for base in (0, 64):
        for d in range(32):
            pm[base + d + 32, base + d] = -1.0       # rot_half: -x2 into top
            pm[base + d, base + d + 32] = 1.0        # +x1 into bottom
    return pm


def causal_masks():
    k = np.arange(P)[:, None]
    q = np.arange(512)[None, :]
    import ml_dtypes
    m = np.stack([(mm * P + k <= q) for mm in range(4)], axis=1)
    return np.ascontiguousarray(m.astype(ml_dtypes.bfloat16))  # [128, 4, 512]


def host_inputs(x, W_qkv, b_qkv, W_proj, b_proj):
    import ml_dtypes
    bf16 = ml_dtypes.bfloat16
    x = np.asarray(x, dtype=np.float32)
    W_qkv = np.asarray(W_qkv, dtype=np.float32)
    b_qkv = np.asarray(b_qkv, dtype=np.float32)
    W_proj = np.asarray(W_proj, dtype=np.float32)
    scale = 1.0 / math.sqrt(D)
    cos2, sin2 = rope_tables()
    cos2 = cos2.astype(bf16)
    masks = causal_masks()
    pm = perm_matrix().astype(bf16)
    in_maps = []
    for core in range(NCORES):
        b = core // 2
        hg = core % 2
        s = hg * DL
        wq = W_qkv[:, s:s + DL] * scale
        wk = W_qkv[:, C + s:C + s + DL]
        wqk_f = np.concatenate([wq, wk], axis=1)                # [1024, 1024]
        # [o*128+p, j*128+n] -> [j, p, o, n]
        wqk_d = np.ascontiguousarray(
            wqk_f.reshape(8, P, 8, P).transpose(2, 1, 0, 3).astype(bf16))
        ord_eo = [0, 2, 4, 6, 1, 3, 5, 7]
        wv_f = W_qkv[:, 2 * C + s:2 * C + s + DL]               # [1024, 512]
        wv_f = wv_f.reshape(C, 8, 64)[:, ord_eo, :].reshape(C, DL)
        wv_d = np.ascontiguousarray(
            wv_f.reshape(8, P, DL).transpose(1, 0, 2).astype(bf16))
        wp_f = W_proj[s:s + DL, :]                              # [512, 1024]
        wp_d = np.ascontiguousarray(
            wp_f.reshape(4, P, C).transpose(1, 0, 2).astype(bf16))
        bq = b_qkv[s:s + DL] * scale
        bk = b_qkv[C + s:C + s + DL]
        bv = b_qkv[2 * C + s:2 * C + s + DL]
        bv = bv.reshape(8, 64)[ord_eo].reshape(DL)
        bqk = np.concatenate([bq, bk]).reshape(8, P).T          # [128, 8]
        bvb = np.tile(bv[None, :], (P, 1))                      # [128, 512]
        bias = np.ascontiguousarray(
            np.concatenate([bqk, bvb], axis=1).astype(np.float32))
        in_maps.append({
            "xb": np.ascontiguousarray(x[b].astype(bf16)),
            "wqk": wqk_d, "wv": wv_d, "wp": wp_d,
            "cos2": cos2, "sin2": sin2, "bias": bias, "mask": masks,
            "perm": pm,
        })
    return in_maps


_NC_CACHE = {}


def run(in_maps, **kwargs):
    if "nc" not in _NC_CACHE:
        _NC_CACHE["nc"] = build_nc()
    return run_bass_kernel_spmd(
        _NC_CACHE["nc"], in_maps, core_ids=list(range(NCORES)), **kwargs)


def kernel(x, W_qkv, b_qkv, W_proj, b_proj, **extra):
    in_maps = host_inputs(x, W_qkv, b_qkv, W_proj, b_proj)
    res = run(in_maps)
    b_proj = np.asarray(b_proj, dtype=np.float32)
    out = np.empty((B, T, C), dtype=np.float32)
    for b in range(B):
        out[b] = (res.results[2 * b]["out"].astype(np.float32)
                  + res.results[2 * b + 1]["out"].astype(np.float32) + b_proj)
    return out
